# revision 22
# baseline (speedup 1.0000x reference)
"""Multi-head causal self-attention (B=4, T=2048, D=1024, H=16) on 8 TRN2 cores.

Sharding (hardcoded): data-parallel over the 4 batches x tensor-parallel over
head halves. Core c handles batch c//2 and local heads (c%2)*8 .. (c%2)*8+7
for all 2048 positions. The host casts x and the weight slices to bf16 and
pre-packs weights into the on-chip layout (one contiguous 8KB row per
partition); it sums the two partial outputs per batch and adds the bias bo.

Per-core schedule: projections and attention interleave so the tensor engine
never starves while the scalar engine (exp) pipelines underneath:

  for g in 0..3:   x positions g*512..g*512+511, query quad qb0 = 4g
    8 DMA-transposes x_d -> xt columns for this group                 (DMA)
    K^T/Q^T/V projections for this position group                     (PE)
    attention quad g (needs only projections <= g):
      per head pair, per key block kb (causal-trimmed columns):
        S^T both parities -> one PSUM tile [128, 2, 512]; the two
        64-contraction matmuls run concurrently in PE row groups      (PE)
        future positions get -1e9 via an accumulated strict-upper
        triangular mask matmul on the diagonal block                  (PE)
        P^T = exp(S^T * 1/8)  (scale folded into the activation)      (ACT)
        ctx^T[par] += V[kb,par] (+ones column) @ P^T[par]             (PE)
      normalize: l rows -> [64,16] reciprocal -> DRAM-bounce
        partition-broadcast -> multiply; parity 1 shifted via DMA
    out projection per query block
"""
import numpy as np
import ml_dtypes

import concourse.bass as bass
import concourse.mybir as mybir
import concourse.tile as tile
from concourse import bacc
from concourse.bass_utils import run_bass_kernel_spmd
from concourse.masks import make_upper_triangular

F32 = mybir.dt.float32
BF16 = mybir.dt.bfloat16
AF = mybir.ActivationFunctionType
BF16NP = ml_dtypes.bfloat16

B, T, D = 4, 2048, 1024
HL = 8              # local heads per core
HP = HL // 2        # local head pairs (two heads share 128 partitions)
DH = 64
PO = D // 128       # contraction chunks over D
CD = HL * DH        # 512: local context feature dim
FC = CD // 128      # 4
NB = T // 128       # 16 query/key blocks of 128
QUAD = 4            # query blocks handled together (512 S^T columns)
SCALE = 1.0 / 8.0   # 1/sqrt(DH)
NEG = -1e9


def _emit_proj_q(nc, tw, xt_sb, mmp, wq_sb, qt_sb):
    """Project Q^T for one 512-column group of x^T (only needs xt)."""
    tsl = slice(tw * 512, (tw + 1) * 512)
    for hp in range(HP):
        ps = mmp.tile([128, 512], F32, tag="mm")
        for po in range(PO):
            nc.tensor.matmul(
                ps, lhsT=wq_sb[:, po, hp * 128:(hp + 1) * 128],
                rhs=xt_sb[:, tw, po, :],
                start=(po == 0), stop=(po == PO - 1),
            )
        nc.vector.tensor_copy(qt_sb[:, hp, tsl], ps)


def _emit_proj_kv(nc, tw, xt_sb, mmp, wk_sb, wv_sb, kt_sb, v_sb):
    """Project K^T and V for one 512-column group of x^T."""
    tsl = slice(tw * 512, (tw + 1) * 512)
    for hp in range(HP):
        ps = mmp.tile([128, 512], F32, tag="mm")
        for po in range(PO):
            nc.tensor.matmul(
                ps, lhsT=wk_sb[:, po, hp * 128:(hp + 1) * 128],
                rhs=xt_sb[:, tw, po, :],
                start=(po == 0), stop=(po == PO - 1),
            )
        nc.vector.tensor_copy(kt_sb[:, hp, tsl], ps)
    for tb in range(4):
        kb = tw * 4 + tb
        ps = mmp.tile([128, 512], F32, tag="mm")
        for po in range(PO):
            nc.tensor.matmul(
                ps, lhsT=xt_sb[:, tw, po, tb * 128:(tb + 1) * 128],
                rhs=wv_sb[:, po, :],
                start=(po == 0), stop=(po == PO - 1),
            )
        nc.vector.tensor_copy(
            v_sb[:, kb, :, 0:64], ps.rearrange("p (h d) -> p h d", h=HL)
        )


def _emit_attn_kbs(nc, qb0, kb_lo, kb_hi, hp, ctx, kt_sb, qt_sb, v_sb,
                   utri01, stp, ptp, last_kb):
    """S^T -> exp -> AV for key blocks kb_lo..kb_hi-1 of one head pair."""
    for kb in range(kb_lo, kb_hi):
        j = kb - qb0
        c0 = max(j, 0) * 128   # first live column in the 512-q window
        st = stp.tile([128, 2, 512], F32, tag="st")
        for par, lo in ((0, 0), (1, 64)):
            nc.tensor.matmul(
                st[:, par, c0:512],
                lhsT=kt_sb[lo:lo + 64, hp, kb * 128:(kb + 1) * 128],
                rhs=qt_sb[lo:lo + 64, hp, qb0 * 128 + c0:(qb0 + QUAD) * 128],
                start=True, stop=True,
            )
        pt = ptp.tile([128, 2, 512], BF16, tag="pt")
        nc.scalar.activation(pt[:, :, c0:512], st[:, :, c0:512],
                             AF.Exp, scale=SCALE)
        if j >= 0:
            # zero future positions in the diagonal block (keep-mask is
            # upper triangular incl. diagonal in (key, query) layout)
            for par in (0, 1):
                nc.vector.tensor_mul(pt[:, par, c0:c0 + 128],
                                     pt[:, par, c0:c0 + 128], utri01)
        for par in (0, 1):
            nc.tensor.matmul(
                ctx[par][:, c0:512],
                lhsT=v_sb[:, kb, 2 * hp + par, :],
                rhs=pt[:, par, c0:512],
                start=(kb == kb_lo), stop=(kb == last_kb),
                skip_group_check=True,
            )


def _emit_quad_head(nc, qb0, kb_hi, kt_sb, qt_sb, v_sb, utri01,
                    stp, ptp, cxp, spillp):
    """First kb_hi key blocks of a quad; partial ctx spilled to SBUF f32."""
    spills = []
    for hp in range(HP):
        ctx = [cxp.tile([65, 512], F32, tag="ctx", name=f"ctx{par}")
               for par in (0, 1)]
        _emit_attn_kbs(nc, qb0, 0, kb_hi, hp, ctx, kt_sb, qt_sb, v_sb,
                       utri01, stp, ptp, kb_hi - 1)
        sp = []
        for par in (0, 1):
            t = spillp.tile([65, 512], F32, tag="spill",
                            name=f"sp{hp}_{par}")
            nc.vector.tensor_copy(t, ctx[par])
            sp.append(t)
        spills.append(sp)
    return spills


def _emit_quad(nc, qb0, kt_sb, qt_sb, v_sb, utri01, ones65,
               stp, ptp, cxp, lvp, ctxt16p, mmp, kb_lo=0, spills=None):
    """Attention for query blocks qb0..qb0+3, all 4 local head pairs.

    Chunk-level pipeline per key block: S^T (both parities into one PSUM
    tile) -> exp -> AV accumulation, so the scalar engine runs one block
    behind the tensor engine. Columns left of the diagonal are never
    computed or consumed (causal trim); the diagonal block gets -1e9 on
    its strict upper triangle via an accumulated mask matmul.
    """
    nkb = qb0 + QUAD
    ctxt16s = []
    for hp in range(HP):
        ctx = [cxp.tile([65, 512], F32, tag="ctx", name=f"ctx{par}")
               for par in (0, 1)]
        _emit_attn_kbs(nc, qb0, kb_lo, nkb, hp, ctx, kt_sb, qt_sb, v_sb,
                       utri01, stp, ptp, nkb - 1)
        # normalize: ctx^T[dh, q] /= l[q] with l in row 64. Broadcast the
        # l row across 65 partitions with a 1-row ones-stationary matmul,
        # then a single-pass approximate reciprocal and a multiply; only
        # the parity-1 partition shift still rides a DMA.
        ctxu = {}
        for par in (0, 1):
            ctxu[par] = lvp.tile([65, 512], BF16, tag="ctxu",
                                 name=f"ctxu{par}")
            if spills is not None:
                nc.vector.tensor_add(ctxu[par], ctx[par], spills[hp][par])
            else:
                nc.vector.tensor_copy(ctxu[par], ctx[par])
        lbinv = {}
        for par in (0, 1):
            ps = mmp.tile([65, 512], F32, tag="mm", name=f"lb{par}")
            nc.tensor.matmul(ps, lhsT=ones65[64:65, :],
                             rhs=ctxu[par][64:65, :], start=True, stop=True)
            lbinv[par] = lvp.tile([65, 512], F32, tag="lbi", name=f"lbi{par}")
            nc.vector.reciprocal_approx_fast(lbinv[par], ps)
        c16 = ctxt16p.tile([128, 512], BF16, tag="c16", name=f"c16_{hp}")
        nc.vector.tensor_mul(c16[0:64, :], ctxu[0][0:64, :], lbinv[0][0:64, :])
        tmp = lvp.tile([64, 512], BF16, tag="ctmp")
        nc.vector.tensor_mul(tmp, ctxu[1][0:64, :], lbinv[1][0:64, :])
        nc.gpsimd.dma_start(c16[64:128, :], tmp)
        ctxt16s.append(c16)
    return ctxt16s


def _emit_out_proj_qb(nc, qb0, qloc, ctxt16s, wo_sb, mmp, osbp, out_d):
    """Output projection for query block qb0 + qloc."""
    qb = qb0 + qloc
    for dw in range(2):
        ps = mmp.tile([128, 512], F32, tag="mm")
        for hp in range(HP):
            nc.tensor.matmul(
                ps, lhsT=ctxt16s[hp][:, qloc * 128:(qloc + 1) * 128],
                rhs=wo_sb[:, hp, dw * 512:(dw + 1) * 512],
                start=(hp == 0), stop=(hp == HP - 1),
            )
        osb = osbp.tile([128, 512], F32, tag="osb")
        if dw == 0:
            nc.vector.tensor_copy(osb, ps)
        else:
            nc.scalar.activation(osb, ps, AF.Copy)
        nc.sync.dma_start(
            out_d[qb * 128:(qb + 1) * 128, dw * 512:(dw + 1) * 512], osb)


def build_nc():
    nc = bacc.Bacc("TRN2", target_bir_lowering=False)
    x_d = nc.dram_tensor("x", [128, PO * T], BF16, kind="ExternalInput")
    wq_d = nc.dram_tensor("wq", [128, PO * CD], BF16, kind="ExternalInput")
    wk_d = nc.dram_tensor("wk", [128, PO * CD], BF16, kind="ExternalInput")
    wv_d = nc.dram_tensor("wv", [128, PO * CD], BF16, kind="ExternalInput")
    wo_d = nc.dram_tensor("wo", [128, FC * D], BF16, kind="ExternalInput")
    out_d = nc.dram_tensor("out", [T, D], F32, kind="ExternalOutput")

    with tile.TileContext(nc) as tc:
        with (
            tc.tile_pool(name="consts", bufs=1) as consts,
            tc.tile_pool(name="wsb", bufs=1) as wsb,
            tc.tile_pool(name="big", bufs=1) as big,
            tc.tile_pool(name="pt", bufs=6) as ptp,
            tc.tile_pool(name="lv", bufs=2) as lvp,
            tc.tile_pool(name="ctxt16", bufs=4) as ctxt16p,
            tc.tile_pool(name="osb", bufs=2) as osbp,
            tc.tile_pool(name="spill", bufs=8) as spillp,
            tc.tile_pool(name="dram", bufs=4, space="DRAM") as dramp,
            tc.tile_pool(name="mm", bufs=2, space="PSUM") as mmp,
            tc.tile_pool(name="st", bufs=2, space="PSUM") as stp,
            tc.tile_pool(name="cx", bufs=2, space="PSUM") as cxp,
        ):
            utri01 = consts.tile([128, 128], BF16, tag="utri01")
            make_upper_triangular(nc, utri01, val=1.0, diag=True)
            ones65 = consts.tile([65, 65], BF16, tag="ones65")
            nc.gpsimd.memset(ones65, 1.0)

            xt_sb = big.tile([128, 4, PO, 512], BF16, tag="xt")
            kt_sb = big.tile([128, HP, T], BF16, tag="kt")
            qt_sb = big.tile([128, HP, T], BF16, tag="qt")
            v_sb = big.tile([128, NB, HL, 65], BF16, tag="v")
            nc.gpsimd.memset(v_sb[:, :, :, 64:65], 1.0)

            # pre-packed bf16 weights: one contiguous row per partition
            wq_sb = wsb.tile([128, PO, CD], BF16, tag="wq")
            wk_sb = wsb.tile([128, PO, CD], BF16, tag="wk")
            wv_sb = wsb.tile([128, PO, CD], BF16, tag="wv")
            wo_sb = wsb.tile([128, FC, D], BF16, tag="wo")
            wk_src = wk_d.rearrange("p (a b) -> p a b", a=PO)
            nc.scalar.dma_start(wk_sb[:, 0:4, :], wk_src[:, 0:4, :])
            nc.gpsimd.dma_start(wk_sb[:, 4:8, :], wk_src[:, 4:8, :])
            wq_src = wq_d.rearrange("p (a b) -> p a b", a=PO)
            nc.scalar.dma_start(wq_sb[:, 0:4, :], wq_src[:, 0:4, :])
            nc.gpsimd.dma_start(wq_sb[:, 4:8, :], wq_src[:, 4:8, :])
            nc.scalar.dma_start(wv_sb, wv_d.rearrange("p (a b) -> p a b", a=PO))
            nc.gpsimd.dma_start(wo_sb, wo_d.rearrange("p (a b) -> p a b", a=FC))

            # interleaved emission: attention quad g follows its projection
            # group; the scheduler fills exp stalls with later projections
            xt_src = x_d.rearrange("p (g a b) -> p g a b", g=4, a=PO)
            for g in range(4):
                nc.sync.dma_start(xt_sb[:, g, :, :], xt_src[:, g, :, :])
            _emit_proj_q(nc, 0, xt_sb, mmp, wq_sb, qt_sb)
            _emit_proj_kv(nc, 0, xt_sb, mmp, wk_sb, wv_sb, kt_sb, v_sb)
            for g in range(1, 4):
                _emit_proj_q(nc, g, xt_sb, mmp, wq_sb, qt_sb)
            # per group: K/V projections, then this quad's diagonal tail
            # (bulk was computed in the previous iteration's head), its out
            # projection, and the next quad's sub-diagonal head whose exp
            # overlaps the next K/V projection group
            spills = None
            for g in range(4):
                if g > 0:
                    _emit_proj_kv(nc, g, xt_sb, mmp, wk_sb, wv_sb, kt_sb,
                                  v_sb)
                qb0 = g * QUAD
                # attention beats projections in the ready-heap so exp is
                # always fed; projections fill the tensor engine's stalls
                with tc.high_priority(offset=2000):
                    ctxt16s = _emit_quad(nc, qb0, kt_sb, qt_sb, v_sb,
                                         utri01, ones65,
                                         stp, ptp, cxp, lvp, ctxt16p, mmp,
                                         kb_lo=(4 * g if g > 0 else 0),
                                         spills=spills)
                for qloc in range(QUAD):
                    _emit_out_proj_qb(nc, qb0, qloc, ctxt16s, wo_sb,
                                      mmp, osbp, out_d)
                if g < 3:
                    with tc.high_priority(offset=2000):
                        spills = _emit_quad_head(nc, (g + 1) * QUAD,
                                                 4 * (g + 1), kt_sb,
                                                 qt_sb, v_sb, utri01,
                                                 stp, ptp, cxp, spillp)

    nc.compile()
    return nc


_CACHE = {}


def _get_nc():
    if "nc" not in _CACHE:
        _CACHE["nc"] = build_nc()
    return _CACHE["nc"]


def _pack_w(w):
    """[128k, N] -> [128, k*N] bf16: partition p holds rows {k*128+p}."""
    k = w.shape[0] // 128
    n = w.shape[1]
    return np.ascontiguousarray(
        w.reshape(k, 128, n).transpose(1, 0, 2).reshape(128, k * n)
    ).astype(BF16NP)


def make_in_maps(x, Wq, Wk, Wv, Wo):
    x = np.asarray(x, np.float32)
    Wq = np.asarray(Wq, np.float32)
    Wk = np.asarray(Wk, np.float32)
    Wv = np.asarray(Wv, np.float32)
    Wo = np.asarray(Wo, np.float32)
    in_maps = []
    for c in range(8):
        b, hh = c // 2, c % 2
        cols = slice(hh * CD, (hh + 1) * CD)
        xt = np.ascontiguousarray(
            x[b].T.reshape(PO, 128, 4, 512).transpose(1, 2, 0, 3)
            .reshape(128, PO * T)
        ).astype(BF16NP)
        in_maps.append({
            "x": xt,
            "wq": _pack_w(Wq[:, cols]),
            "wk": _pack_w(Wk[:, cols]),
            "wv": _pack_w(Wv[:, cols]),
            "wo": _pack_w(Wo[cols, :]),
        })
    return in_maps


def gather_output(results, bo):
    bo = np.asarray(bo, np.float32)
    out = np.empty((B, T, D), np.float32)
    for b in range(B):
        out[b] = results[2 * b]["out"] + results[2 * b + 1]["out"] + bo[None, :]
    return out


def kernel(x, Wq, Wk, Wv, Wo, bo):
    nc = _get_nc()
    in_maps = make_in_maps(x, Wq, Wk, Wv, Wo)
    res = run_bass_kernel_spmd(nc, in_maps, core_ids=list(range(8)))
    return gather_output(res.results, bo)
